# revision 23
# baseline (speedup 1.0000x reference)
"""MACCL loss kernel for Trainium2 (8 NeuronCores, SPMD data-parallel).

Strategy (v2)
-------------
The O(B^2 D) contrastive part dominates (B=8192, D=256).  The host does
the O(B*D) data prep that used to run on-device (and was the pipeline
bottleneck): permute rows label-0-first, compute row norms, quantize the
transposed features to fp8(e4m3) in the [K=128, 2, B] DoubleRow layout.
Each core then only runs the O(B^2) part:

  - 8x 1024-wide fp8 DoubleRow matmuls per 2048-column group: one
    instruction contracts the full K=256 (two k-tiles packed), so the
    PE does 0.5 cyc/row and the whole sim matrix costs ~14us/core.
  - ACT exp(scale_i * psum) in place, scale = r_i/T as a per-partition
    AP (the row normalization folds into the activation for free), with
    accum_out producing per-label-segment row sums (columns are
    label-sorted, segments are contiguous).
  - The diagonal term is reproduced bit-exactly by a DR matmul over the
    core's own (lhsT unscaled, rhs r_j-scaled) fp8 slices -- the same
    host arrays the main loop consumes -- so pos_sum = S_same - d
    cancels exactly on the host.

Per-core output: stats [128, 24] = {S0, S1, exp(diag)} x 8 row tiles.
Host finalizes in f64 (center/margin/sigma/log/mean) exactly mirroring
the reference formulas; norms/rowsums for the center and sigma terms
are host-side f64 (more accurate than the reference's own f32).

The operand quantization error only touches r_con, which is ~0.3% of
the total loss, so fp8 keeps the end-to-end error ~1e-5.
"""

import os
import sys

for _p in ("/root/.axon_site", "/root/.axon_site/_ro/trn_rl_repo",
           "/root/.axon_site/_ro/pypackages", "/opt/trn_rl_repo", "/opt/pypackages"):
    if os.path.isdir(_p) and _p not in sys.path:
        sys.path.append(_p)

import numpy as np
import ml_dtypes
from contextlib import ExitStack

import concourse.bass as bass
import concourse.bacc as bacc
import concourse.tile as tile
from concourse import mybir
from concourse.bass_utils import run_bass_kernel_spmd

F32 = mybir.dt.float32
BF16 = mybir.dt.bfloat16
F8 = mybir.dt.float8e4

P = 128
D = 256
B = 8192
NCORES = 8
BPC = B // NCORES
MROW = BPC // P
GW = 2048
NG = B // GW
TEMPERATURE = 0.07
MARGIN_BASE = 0.5
LAMBDA_SIGMA = 0.3
LAMBDA_RESOLUTION = 0.3
RESOLUTION_RATIO = 224.0 / 900.0
ALPHA, BETA, GAMMA = 1.0, 1.0, 0.5

# "f8dr" = fp8 DoubleRow (1 matmul per chunk, 0.5 cyc/row)
# "bf16" = bf16 with explicit k-loop (fallback)
MODE = os.environ.get("MACCL_MODE", "f8dr")
# moving-operand width per matmul (psum bank = 512 f32; DoubleRow N=1024
# fails the walrus s3d3_mm_num_elements ISA check, so 512 it is)
N_MM = int(os.environ.get("MACCL_N_MM", "512"))
# dummy LDWEIGHTS per main-loop iteration (HAM warming experiment): measured
# SLOWER on HW (112.0us vs 101.3us at 0) -- they land on the PE critical
# path instead of warming the clock. Keep 0.
N_WARM = int(os.environ.get("MACCL_N_WARM", "0"))


def _group_plan(n0, gw=GW):
    """One accumulator slot per gw-wide group (assigned the label of its
    majority side); the boundary group's minority sub-range is reduced
    separately on DVE and add/subtracted on the host.

    Returns (k0, boundary): k0 = #groups assigned label 0 (a contiguous
    prefix), boundary = None or (g, rs, re, side_label) for the minority
    sub-range (group-local columns)."""
    ng = B // gw
    k0 = 0
    boundary = None
    for g in range(ng):
        lo, hi = g * gw, (g + 1) * gw
        if n0 >= hi:
            k0 += 1
        elif n0 > lo:
            left, right = n0 - lo, hi - n0
            if left >= right:
                k0 += 1                       # group counts as label 0
                boundary = (g, n0 - lo, gw, 1)
            else:
                boundary = (g, 0, n0 - lo, 0)
    return k0, boundary


def build_program(n0, mode=MODE):
    use_f8 = mode == "f8dr"
    op_dt = F8 if use_f8 else BF16
    perf = mybir.MatmulPerfMode.DoubleRow if use_f8 else None

    k0, boundary = _group_plan(n0)
    nslots = NG
    k1 = nslots - k0

    AX = mybir.AxisListType.X
    AF = mybir.ActivationFunctionType

    nc = bacc.Bacc("TRN2", target_bir_lowering=False, debug=False,
                   num_devices=NCORES)
    a8_d = nc.dram_tensor("a8", [P, 2, B], op_dt, kind="ExternalInput").ap()
    mm8_d = nc.dram_tensor("mm8", [P, 2, BPC], op_dt, kind="ExternalInput").ap()
    rot_d = nc.dram_tensor("rot", [P, MROW], F32, kind="ExternalInput").ap()
    # stats: [S0_dev | S1_dev | boundary-minority partial] per row tile
    stats_d = nc.dram_tensor("stats", [P, 3 * MROW], F32, kind="ExternalOutput").ap()

    with tile.TileContext(nc) as tc, ExitStack() as ctx:
        singles = ctx.enter_context(tc.tile_pool(name="singles", bufs=1))
        ps_pool = ctx.enter_context(tc.tile_pool(name="ps", bufs=1, space="PSUM"))

        a8_sb = singles.tile([P, 2, B], op_dt)
        m8_sb = singles.tile([P, 2, BPC], op_dt)
        rot_sb = singles.tile([P, MROW], F32)
        stats_sb = singles.tile([P, 3 * MROW], F32)
        accs_t = singles.tile([P, MROW * nslots], F32)
        prime = singles.tile([P, 1], F32)

        # Priming activation with no input deps (scale=0 ignores the garbage
        # read): hoists the ~1.5us ACT table load into the DMA lead-in.
        nc.scalar.activation(prime, prime, AF.Exp, scale=0.0)

        # packed DMAs for the small operands, then a8 chunk 0 (which gates
        # the first matmuls), then the rest
        nc.sync.dma_start(m8_sb, mm8_d)
        nc.sync.dma_start(rot_sb, rot_d)
        for g in range(NG):
            nc.sync.dma_start(a8_sb[:, :, g * GW:(g + 1) * GW],
                              a8_d[:, :, g * GW:(g + 1) * GW])

        if boundary is None:
            nc.vector.memset(stats_sb[:, 2 * MROW:3 * MROW], 0.0)

        psA = ps_pool.tile([P, GW], F32, name="psA")
        psB = ps_pool.tile([P, GW], F32, name="psB")
        pss = [psA, psB]
        # EXP lands in SBUF bf16; the per-group row sums run on DVE (2x mode
        # on 2-byte data) instead of the ACT accumulator, saving the ~210ns
        # ACTIVATION_READ_ACCUMULATOR after every EXP.
        eA = singles.tile([P, GW], BF16, name="eA")
        eB = singles.tile([P, GW], BF16, name="eB")
        ees = [eA, eB]

        def mm(out_ap, lhsT, rhs_tile, c0, n):
            if use_f8:
                nc.tensor.matmul(out_ap, lhsT, rhs_tile[:, :, c0:c0 + n],
                                 start=True, stop=True, perf_mode=perf)
            else:
                for k in (0, 1):
                    nc.tensor.matmul(out_ap, lhsT[:, k, :],
                                     rhs_tile[:, k, c0:c0 + n],
                                     start=(k == 0), stop=(k == 1))

        def accs(m):
            return accs_t[:, m * nslots:(m + 1) * nslots]

        # ---- main loop: per column group, per own row tile; one EXP +
        # accumulator per group, minority sub-range reduced on DVE ----
        nb = 0
        for g in range(NG):
            for m in range(MROW):
                psg = pss[nb]
                eg = ees[nb]; nb ^= 1
                lhsT = m8_sb[:, :, m * P:(m + 1) * P]
                for s in range(GW // N_MM):
                    mm(psg[:, s * N_MM:(s + 1) * N_MM], lhsT, a8_sb,
                       g * GW + s * N_MM, N_MM)
                nc.scalar.activation(eg, psg, AF.Exp,
                                     scale=rot_sb[:, m:m + 1])
                nc.vector.reduce_sum(accs(m)[:, g:g + 1], eg, axis=AX)
                if boundary is not None and boundary[0] == g:
                    _, rs, re, _side = boundary
                    nc.vector.reduce_sum(
                        stats_sb[:, 2 * MROW + m:2 * MROW + m + 1],
                        eg[:, rs:re], axis=AX)

        # ---- per-row-tile S0/S1 (device-side partials) ----
        for m in range(MROW):
            s0 = stats_sb[:, m:m + 1]
            s1 = stats_sb[:, MROW + m:MROW + m + 1]
            if k0 > 0:
                nc.vector.reduce_sum(s0, accs(m)[:, 0:k0], axis=AX)
            else:
                nc.vector.memset(s0, 0.0)
            if k1 > 0:
                nc.vector.reduce_sum(s1, accs(m)[:, k0:nslots], axis=AX)
            else:
                nc.vector.memset(s1, 0.0)

        nc.sync.dma_start(stats_d, stats_sb)

    nc.compile()
    return nc


_PROGRAM_CACHE = {}


def _get_program(n0):
    key = (n0, MODE, N_MM)
    if key not in _PROGRAM_CACHE:
        _PROGRAM_CACHE[key] = build_program(n0)
    return _PROGRAM_CACHE[key]


def run_device(features, labels, trace=False):
    """Host prep + 8-core device run.  Returns (stats dict aligned to the
    label-sorted permutation, permutation order, n0, raw results)."""
    Bq, d = features.shape
    assert d == D and Bq == B

    order = np.argsort(labels, kind="stable")
    n0 = int((labels == 0).sum())
    fp = np.ascontiguousarray(features[order]).astype(np.float32, copy=False)

    # host-side O(B*D) prep
    fp64 = fp.astype(np.float64)
    norms2 = (fp64 * fp64).sum(axis=1)                  # [B]
    rowsum = fp64.sum(axis=1)                           # [B]
    r = 1.0 / np.maximum(np.sqrt(norms2), 1e-12)        # [B]
    r32 = r.astype(np.float32)

    op_np = ml_dtypes.float8_e4m3 if MODE == "f8dr" else ml_dtypes.bfloat16
    # [K=128, 2, B] DoubleRow layout: D index = ktile*128 + partition
    fT = np.ascontiguousarray(fp.T).reshape(2, P, B).transpose(1, 0, 2)
    m8_full = np.ascontiguousarray(fT).astype(op_np)                 # unscaled
    a8 = np.ascontiguousarray(fT * r32[None, None, :]).astype(op_np)  # scaled

    rot_full = (r32 / np.float32(TEMPERATURE)).astype(np.float32)

    nc = _get_program(n0)
    in_maps = []
    for c in range(NCORES):
        sl = slice(c * BPC, (c + 1) * BPC)
        in_maps.append({"a8": a8,
                        "mm8": np.ascontiguousarray(m8_full[:, :, sl]),
                        "rot": np.ascontiguousarray(
                            rot_full[sl].reshape(MROW, P).T)})
    res = run_bass_kernel_spmd(nc, in_maps, list(range(NCORES)), trace=trace)

    parts = []
    for c in range(NCORES):
        st = res.results[c]["stats"]          # [128, 3*MROW]
        arr = st.reshape(P, 3, MROW).transpose(1, 2, 0).reshape(3, BPC)
        parts.append(arr)
    full = np.concatenate(parts, axis=1)      # [3, B] in permuted row order
    S0, S1, small = (full[0].astype(np.float64),
                     full[1].astype(np.float64),
                     full[2].astype(np.float64))
    _, boundary = _group_plan(n0)
    if boundary is not None:
        side = boundary[3]
        if side == 0:
            S0, S1 = S0 + small, S1 - small
        else:
            S0, S1 = S0 - small, S1 + small

    # host-side diagonal: exp of the same quantized self-product the device
    # sums into S_same, rounded to bf16 exactly as the device EXP output is,
    # so the S_same - d cancellation tracks the device values.
    q = np.einsum("pkj,pkj->j", m8_full.astype(np.float64),
                  a8.astype(np.float64))
    d = np.exp(q * rot_full.astype(np.float64))
    d = np.asarray(d, dtype=ml_dtypes.bfloat16).astype(np.float64)

    stats = {"norms2": norms2, "rowsum": rowsum, "S0": S0, "S1": S1, "d": d}
    return stats, order, n0, res


def finalize(stats, order, n0, labels, normal_center, running_sigma, B):
    """Host O(B) finalization mirroring the reference formulas (float64)."""
    labels_p = labels[order]
    nmf = (labels_p == 0)
    amf = (labels_p == 1)
    norms2 = stats["norms2"].astype(np.float64)
    rowsum = stats["rowsum"].astype(np.float64)
    S0 = stats["S0"].astype(np.float64)
    S1 = stats["S1"].astype(np.float64)
    ddiag = stats["d"].astype(np.float64)

    c = np.asarray(normal_center, dtype=np.float64)
    csq = float((c * c).sum())
    if csq != 0.0:
        raise NotImplementedError  # caller routes to the general-center path
    dist_sq = norms2  # center == 0
    n_normal = float(nmf.sum())

    with np.errstate(divide="ignore", invalid="ignore"):
        n_el = n_normal * D
        masked_sum = float((rowsum * nmf).sum())
        mean = masked_sum / n_el
        sum_sq_m = float((norms2 * nmf).sum())
        var = (sum_sq_m - 2.0 * mean * masked_sum + mean * mean * n_el) / (n_el - 1.0)
        sigma_new = 0.9 * float(running_sigma) + 0.1 * np.sqrt(var)

        m_adaptive = (MARGIN_BASE + LAMBDA_SIGMA * sigma_new
                      + LAMBDA_RESOLUTION * (1.0 - RESOLUTION_RATIO))
        dist = np.sqrt(np.maximum(dist_sq, 0.0))
        r_center = dist_sq * nmf
        r_margin = np.maximum(m_adaptive - dist, 0.0) * amf

        S_same = np.where(nmf, S0, S1)
        S_diff = np.where(nmf, S1, S0)
        pos_sum = S_same - ddiag
        neg_sum = S_diff
        n1 = B - n0
        cnt_pos = np.where(nmf, n0 - 1, n1 - 1)
        cnt_neg = np.where(nmf, n1, n0)
        has_both = (cnt_pos > 0) & (cnt_neg > 0)
        pos_safe = np.where(has_both, np.maximum(pos_sum, 1e-12), 1.0)
        den_safe = np.where(has_both, pos_sum + neg_sum + 1e-8, 1.0)
        r_con = np.where(has_both, -np.log(pos_safe / den_safe), 0.0)

        raw_total = ALPHA * r_center + BETA * r_margin + GAMMA * r_con
        total = raw_total.mean()
    return np.array(total, dtype=np.float32)


def _finalize_general_center(stats, order, n0, labels, normal_center,
                             running_sigma, B, features):
    """Fallback for a nonzero normal_center (not hit for spec inputs)."""
    labels_p = labels[order]
    fp = features[order].astype(np.float64)
    c = np.asarray(normal_center, dtype=np.float64)
    qc = fp @ c
    norms2 = stats["norms2"].astype(np.float64)
    dist_sq = norms2 - 2.0 * qc + float((c * c).sum())
    nmf = (labels_p == 0)
    amf = (labels_p == 1)
    rowsum = stats["rowsum"].astype(np.float64)
    S0 = stats["S0"].astype(np.float64)
    S1 = stats["S1"].astype(np.float64)
    ddiag = stats["d"].astype(np.float64)
    n_normal = float(nmf.sum())
    with np.errstate(divide="ignore", invalid="ignore"):
        n_el = n_normal * D
        masked_sum = float((rowsum * nmf).sum())
        mean = masked_sum / n_el
        sum_sq_m = float((norms2 * nmf).sum())
        var = (sum_sq_m - 2.0 * mean * masked_sum + mean * mean * n_el) / (n_el - 1.0)
        sigma_new = 0.9 * float(running_sigma) + 0.1 * np.sqrt(var)
        m_adaptive = (MARGIN_BASE + LAMBDA_SIGMA * sigma_new
                      + LAMBDA_RESOLUTION * (1.0 - RESOLUTION_RATIO))
        dist = np.sqrt(np.maximum(dist_sq, 0.0))
        r_center = dist_sq * nmf
        r_margin = np.maximum(m_adaptive - dist, 0.0) * amf
        S_same = np.where(nmf, S0, S1)
        S_diff = np.where(nmf, S1, S0)
        pos_sum = S_same - ddiag
        neg_sum = S_diff
        n1 = B - n0
        cnt_pos = np.where(nmf, n0 - 1, n1 - 1)
        cnt_neg = np.where(nmf, n1, n0)
        has_both = (cnt_pos > 0) & (cnt_neg > 0)
        pos_safe = np.where(has_both, np.maximum(pos_sum, 1e-12), 1.0)
        den_safe = np.where(has_both, pos_sum + neg_sum + 1e-8, 1.0)
        r_con = np.where(has_both, -np.log(pos_safe / den_safe), 0.0)
        total = (ALPHA * r_center + BETA * r_margin + GAMMA * r_con).mean()
    return np.array(total, dtype=np.float32)


def kernel(features, labels, normal_center, running_sigma):
    features = np.asarray(features, dtype=np.float32)
    labels = np.asarray(labels, dtype=np.int32)
    normal_center = np.asarray(normal_center, dtype=np.float32)
    running_sigma = np.float32(np.asarray(running_sigma))
    Bq = features.shape[0]

    stats, order, n0, _res = run_device(features, labels)
    if float((np.asarray(normal_center, np.float64) ** 2).sum()) != 0.0:
        return _finalize_general_center(stats, order, n0, labels,
                                        normal_center, running_sigma, Bq,
                                        features)
    return finalize(stats, order, n0, labels, normal_center, running_sigma, Bq)


# revision 25
# speedup vs baseline: 1.0740x; 1.0740x over previous
"""MACCL loss kernel for Trainium2 (8 NeuronCores, SPMD data-parallel).

Strategy (v2)
-------------
The O(B^2 D) contrastive part dominates (B=8192, D=256).  The host does
the O(B*D) data prep that used to run on-device (and was the pipeline
bottleneck): permute rows label-0-first, compute row norms, quantize the
transposed features to fp8(e4m3) in the [K=128, 2, B] DoubleRow layout.
Each core then only runs the O(B^2) part:

  - 8x 1024-wide fp8 DoubleRow matmuls per 2048-column group: one
    instruction contracts the full K=256 (two k-tiles packed), so the
    PE does 0.5 cyc/row and the whole sim matrix costs ~14us/core.
  - ACT exp(scale_i * psum) in place, scale = r_i/T as a per-partition
    AP (the row normalization folds into the activation for free), with
    accum_out producing per-label-segment row sums (columns are
    label-sorted, segments are contiguous).
  - The diagonal term is reproduced bit-exactly by a DR matmul over the
    core's own (lhsT unscaled, rhs r_j-scaled) fp8 slices -- the same
    host arrays the main loop consumes -- so pos_sum = S_same - d
    cancels exactly on the host.

Per-core output: stats [128, 24] = {S0, S1, exp(diag)} x 8 row tiles.
Host finalizes in f64 (center/margin/sigma/log/mean) exactly mirroring
the reference formulas; norms/rowsums for the center and sigma terms
are host-side f64 (more accurate than the reference's own f32).

The operand quantization error only touches r_con, which is ~0.3% of
the total loss, so fp8 keeps the end-to-end error ~1e-5.
"""

import os
import sys

for _p in ("/root/.axon_site", "/root/.axon_site/_ro/trn_rl_repo",
           "/root/.axon_site/_ro/pypackages", "/opt/trn_rl_repo", "/opt/pypackages"):
    if os.path.isdir(_p) and _p not in sys.path:
        sys.path.append(_p)

import numpy as np
import ml_dtypes
from contextlib import ExitStack

import concourse.bass as bass
import concourse.bacc as bacc
import concourse.tile as tile
from concourse import mybir
from concourse.bass_utils import run_bass_kernel_spmd

F32 = mybir.dt.float32
BF16 = mybir.dt.bfloat16
F8 = mybir.dt.float8e4

P = 128
D = 256
B = 8192
NCORES = 8
BPC = B // NCORES
MROW = BPC // P
GW = 2048
NG = B // GW
TEMPERATURE = 0.07
MARGIN_BASE = 0.5
LAMBDA_SIGMA = 0.3
LAMBDA_RESOLUTION = 0.3
RESOLUTION_RATIO = 224.0 / 900.0
ALPHA, BETA, GAMMA = 1.0, 1.0, 0.5

# "f8dr" = fp8 DoubleRow (1 matmul per chunk, 0.5 cyc/row)
# "bf16" = bf16 with explicit k-loop (fallback)
MODE = os.environ.get("MACCL_MODE", "f8dr")
# moving-operand width per matmul (psum bank = 512 f32; DoubleRow N=1024
# fails the walrus s3d3_mm_num_elements ISA check, so 512 it is)
N_MM = int(os.environ.get("MACCL_N_MM", "512"))
# dummy LDWEIGHTS per main-loop iteration (HAM warming experiment): measured
# SLOWER on HW (112.0us vs 101.3us at 0) -- they land on the PE critical
# path instead of warming the clock. Keep 0.
N_WARM = int(os.environ.get("MACCL_N_WARM", "0"))


def _group_plan(n0, gw=GW):
    """One accumulator slot per gw-wide group (assigned the label of its
    majority side); the boundary group's minority sub-range is reduced
    separately on DVE and add/subtracted on the host.

    Returns (k0, boundary): k0 = #groups assigned label 0 (a contiguous
    prefix), boundary = None or (g, rs, re, side_label) for the minority
    sub-range (group-local columns)."""
    ng = B // gw
    k0 = 0
    boundary = None
    for g in range(ng):
        lo, hi = g * gw, (g + 1) * gw
        if n0 >= hi:
            k0 += 1
        elif n0 > lo:
            left, right = n0 - lo, hi - n0
            if left >= right:
                k0 += 1                       # group counts as label 0
                boundary = (g, n0 - lo, gw, 1)
            else:
                boundary = (g, 0, n0 - lo, 0)
    return k0, boundary


def build_program(n0, mode=MODE):
    use_f8 = mode == "f8dr"
    op_dt = F8 if use_f8 else BF16
    perf = mybir.MatmulPerfMode.DoubleRow if use_f8 else None

    k0, boundary = _group_plan(n0)
    nslots = NG
    k1 = nslots - k0

    AX = mybir.AxisListType.X
    AF = mybir.ActivationFunctionType

    nc = bacc.Bacc("TRN2", target_bir_lowering=False, debug=False,
                   num_devices=NCORES)
    a8_d = nc.dram_tensor("a8", [P, 2, B], op_dt, kind="ExternalInput").ap()
    mm8_d = nc.dram_tensor("mm8", [P, 2, BPC], op_dt, kind="ExternalInput").ap()
    rot_d = nc.dram_tensor("rot", [P, MROW], F32, kind="ExternalInput").ap()
    # stats: [S0_dev | S1_dev | boundary-minority partial] per row tile
    stats_d = nc.dram_tensor("stats", [P, 3 * MROW], F32, kind="ExternalOutput").ap()

    with tile.TileContext(nc) as tc, ExitStack() as ctx:
        singles = ctx.enter_context(tc.tile_pool(name="singles", bufs=1))
        ps_pool = ctx.enter_context(tc.tile_pool(name="ps", bufs=1, space="PSUM"))

        a8_sb = singles.tile([P, 2, B], op_dt)
        m8_sb = singles.tile([P, 2, BPC], op_dt)
        rot_sb = singles.tile([P, MROW], F32)
        stats_sb = singles.tile([P, 3 * MROW], F32)
        accs_t = singles.tile([P, MROW * nslots], F32)
        prime = singles.tile([P, 1], F32)

        # Priming activation with no input deps (scale=0 ignores the garbage
        # read): hoists the ~1.5us ACT table load into the DMA lead-in.
        nc.scalar.activation(prime, prime, AF.Exp, scale=0.0)

        # packed DMAs for the small operands, then a8 (which gates the first
        # matmuls).  Group 0 streams in N_MM-wide strips so the first matmul
        # starts after ~1/4 of the chunk instead of the whole 0.5MB.
        nc.sync.dma_start(m8_sb, mm8_d)
        nc.sync.dma_start(rot_sb, rot_d)
        for s in range(GW // N_MM):
            nc.sync.dma_start(a8_sb[:, :, s * N_MM:(s + 1) * N_MM],
                              a8_d[:, :, s * N_MM:(s + 1) * N_MM])
        for g in range(1, NG):
            nc.sync.dma_start(a8_sb[:, :, g * GW:(g + 1) * GW],
                              a8_d[:, :, g * GW:(g + 1) * GW])

        if boundary is None:
            nc.vector.memset(stats_sb[:, 2 * MROW:3 * MROW], 0.0)

        psA = ps_pool.tile([P, GW], F32, name="psA")
        psB = ps_pool.tile([P, GW], F32, name="psB")
        pss = [psA, psB]

        def mm(out_ap, lhsT, rhs_tile, c0, n):
            if use_f8:
                nc.tensor.matmul(out_ap, lhsT, rhs_tile[:, :, c0:c0 + n],
                                 start=True, stop=True, perf_mode=perf)
            else:
                for k in (0, 1):
                    nc.tensor.matmul(out_ap, lhsT[:, k, :],
                                     rhs_tile[:, k, c0:c0 + n],
                                     start=(k == 0), stop=(k == 1))

        def accs(m):
            return accs_t[:, m * nslots:(m + 1) * nslots]

        # ---- main loop: per column group, per own row tile; one EXP +
        # accumulator per group, minority sub-range reduced on DVE ----
        nb = 0
        for g in range(NG):
            for m in range(MROW):
                psg = pss[nb]; nb ^= 1
                lhsT = m8_sb[:, :, m * P:(m + 1) * P]
                for s in range(GW // N_MM):
                    mm(psg[:, s * N_MM:(s + 1) * N_MM], lhsT, a8_sb,
                       g * GW + s * N_MM, N_MM)
                nc.scalar.activation(
                    psg, psg, AF.Exp, scale=rot_sb[:, m:m + 1],
                    accum_out=accs(m)[:, g:g + 1])
                if boundary is not None and boundary[0] == g:
                    _, rs, re, _side = boundary
                    nc.vector.reduce_sum(
                        stats_sb[:, 2 * MROW + m:2 * MROW + m + 1],
                        psg[:, rs:re], axis=AX)

        # ---- per-row-tile S0/S1 (device-side partials) ----
        for m in range(MROW):
            s0 = stats_sb[:, m:m + 1]
            s1 = stats_sb[:, MROW + m:MROW + m + 1]
            if k0 > 0:
                nc.vector.reduce_sum(s0, accs(m)[:, 0:k0], axis=AX)
            else:
                nc.vector.memset(s0, 0.0)
            if k1 > 0:
                nc.vector.reduce_sum(s1, accs(m)[:, k0:nslots], axis=AX)
            else:
                nc.vector.memset(s1, 0.0)

        nc.sync.dma_start(stats_d, stats_sb)

    nc.compile()
    return nc


_PROGRAM_CACHE = {}


def _get_program(n0):
    key = (n0, MODE, N_MM)
    if key not in _PROGRAM_CACHE:
        _PROGRAM_CACHE[key] = build_program(n0)
    return _PROGRAM_CACHE[key]


def run_device(features, labels, trace=False):
    """Host prep + 8-core device run.  Returns (stats dict aligned to the
    label-sorted permutation, permutation order, n0, raw results)."""
    Bq, d = features.shape
    assert d == D and Bq == B

    order = np.argsort(labels, kind="stable")
    n0 = int((labels == 0).sum())
    fp = np.ascontiguousarray(features[order]).astype(np.float32, copy=False)

    # host-side O(B*D) prep
    fp64 = fp.astype(np.float64)
    norms2 = (fp64 * fp64).sum(axis=1)                  # [B]
    rowsum = fp64.sum(axis=1)                           # [B]
    r = 1.0 / np.maximum(np.sqrt(norms2), 1e-12)        # [B]
    r32 = r.astype(np.float32)

    op_np = ml_dtypes.float8_e4m3 if MODE == "f8dr" else ml_dtypes.bfloat16
    # [K=128, 2, B] DoubleRow layout: D index = ktile*128 + partition
    fT = np.ascontiguousarray(fp.T).reshape(2, P, B).transpose(1, 0, 2)
    m8_full = np.ascontiguousarray(fT).astype(op_np)                 # unscaled
    a8 = np.ascontiguousarray(fT * r32[None, None, :]).astype(op_np)  # scaled

    rot_full = (r32 / np.float32(TEMPERATURE)).astype(np.float32)

    nc = _get_program(n0)
    in_maps = []
    for c in range(NCORES):
        sl = slice(c * BPC, (c + 1) * BPC)
        in_maps.append({"a8": a8,
                        "mm8": np.ascontiguousarray(m8_full[:, :, sl]),
                        "rot": np.ascontiguousarray(
                            rot_full[sl].reshape(MROW, P).T)})
    res = run_bass_kernel_spmd(nc, in_maps, list(range(NCORES)), trace=trace)

    parts = []
    for c in range(NCORES):
        st = res.results[c]["stats"]          # [128, 3*MROW]
        arr = st.reshape(P, 3, MROW).transpose(1, 2, 0).reshape(3, BPC)
        parts.append(arr)
    full = np.concatenate(parts, axis=1)      # [3, B] in permuted row order
    S0, S1, small = (full[0].astype(np.float64),
                     full[1].astype(np.float64),
                     full[2].astype(np.float64))
    _, boundary = _group_plan(n0)
    if boundary is not None:
        side = boundary[3]
        if side == 0:
            S0, S1 = S0 + small, S1 - small
        else:
            S0, S1 = S0 - small, S1 + small

    # host-side diagonal: exp of the same quantized self-product the device
    # sums into S_same (float64 recompute; rel diff ~1e-5 of d, which is
    # ~0.4% of pos_sum after cancellation -- far inside tolerance)
    q = np.einsum("pkj,pkj->j", m8_full.astype(np.float64),
                  a8.astype(np.float64))
    d = np.exp(q * rot_full.astype(np.float64))

    stats = {"norms2": norms2, "rowsum": rowsum, "S0": S0, "S1": S1, "d": d}
    return stats, order, n0, res


def finalize(stats, order, n0, labels, normal_center, running_sigma, B):
    """Host O(B) finalization mirroring the reference formulas (float64)."""
    labels_p = labels[order]
    nmf = (labels_p == 0)
    amf = (labels_p == 1)
    norms2 = stats["norms2"].astype(np.float64)
    rowsum = stats["rowsum"].astype(np.float64)
    S0 = stats["S0"].astype(np.float64)
    S1 = stats["S1"].astype(np.float64)
    ddiag = stats["d"].astype(np.float64)

    c = np.asarray(normal_center, dtype=np.float64)
    csq = float((c * c).sum())
    if csq != 0.0:
        raise NotImplementedError  # caller routes to the general-center path
    dist_sq = norms2  # center == 0
    n_normal = float(nmf.sum())

    with np.errstate(divide="ignore", invalid="ignore"):
        n_el = n_normal * D
        masked_sum = float((rowsum * nmf).sum())
        mean = masked_sum / n_el
        sum_sq_m = float((norms2 * nmf).sum())
        var = (sum_sq_m - 2.0 * mean * masked_sum + mean * mean * n_el) / (n_el - 1.0)
        sigma_new = 0.9 * float(running_sigma) + 0.1 * np.sqrt(var)

        m_adaptive = (MARGIN_BASE + LAMBDA_SIGMA * sigma_new
                      + LAMBDA_RESOLUTION * (1.0 - RESOLUTION_RATIO))
        dist = np.sqrt(np.maximum(dist_sq, 0.0))
        r_center = dist_sq * nmf
        r_margin = np.maximum(m_adaptive - dist, 0.0) * amf

        S_same = np.where(nmf, S0, S1)
        S_diff = np.where(nmf, S1, S0)
        pos_sum = S_same - ddiag
        neg_sum = S_diff
        n1 = B - n0
        cnt_pos = np.where(nmf, n0 - 1, n1 - 1)
        cnt_neg = np.where(nmf, n1, n0)
        has_both = (cnt_pos > 0) & (cnt_neg > 0)
        pos_safe = np.where(has_both, np.maximum(pos_sum, 1e-12), 1.0)
        den_safe = np.where(has_both, pos_sum + neg_sum + 1e-8, 1.0)
        r_con = np.where(has_both, -np.log(pos_safe / den_safe), 0.0)

        raw_total = ALPHA * r_center + BETA * r_margin + GAMMA * r_con
        total = raw_total.mean()
    return np.array(total, dtype=np.float32)


def _finalize_general_center(stats, order, n0, labels, normal_center,
                             running_sigma, B, features):
    """Fallback for a nonzero normal_center (not hit for spec inputs)."""
    labels_p = labels[order]
    fp = features[order].astype(np.float64)
    c = np.asarray(normal_center, dtype=np.float64)
    qc = fp @ c
    norms2 = stats["norms2"].astype(np.float64)
    dist_sq = norms2 - 2.0 * qc + float((c * c).sum())
    nmf = (labels_p == 0)
    amf = (labels_p == 1)
    rowsum = stats["rowsum"].astype(np.float64)
    S0 = stats["S0"].astype(np.float64)
    S1 = stats["S1"].astype(np.float64)
    ddiag = stats["d"].astype(np.float64)
    n_normal = float(nmf.sum())
    with np.errstate(divide="ignore", invalid="ignore"):
        n_el = n_normal * D
        masked_sum = float((rowsum * nmf).sum())
        mean = masked_sum / n_el
        sum_sq_m = float((norms2 * nmf).sum())
        var = (sum_sq_m - 2.0 * mean * masked_sum + mean * mean * n_el) / (n_el - 1.0)
        sigma_new = 0.9 * float(running_sigma) + 0.1 * np.sqrt(var)
        m_adaptive = (MARGIN_BASE + LAMBDA_SIGMA * sigma_new
                      + LAMBDA_RESOLUTION * (1.0 - RESOLUTION_RATIO))
        dist = np.sqrt(np.maximum(dist_sq, 0.0))
        r_center = dist_sq * nmf
        r_margin = np.maximum(m_adaptive - dist, 0.0) * amf
        S_same = np.where(nmf, S0, S1)
        S_diff = np.where(nmf, S1, S0)
        pos_sum = S_same - ddiag
        neg_sum = S_diff
        n1 = B - n0
        cnt_pos = np.where(nmf, n0 - 1, n1 - 1)
        cnt_neg = np.where(nmf, n1, n0)
        has_both = (cnt_pos > 0) & (cnt_neg > 0)
        pos_safe = np.where(has_both, np.maximum(pos_sum, 1e-12), 1.0)
        den_safe = np.where(has_both, pos_sum + neg_sum + 1e-8, 1.0)
        r_con = np.where(has_both, -np.log(pos_safe / den_safe), 0.0)
        total = (ALPHA * r_center + BETA * r_margin + GAMMA * r_con).mean()
    return np.array(total, dtype=np.float32)


def kernel(features, labels, normal_center, running_sigma):
    features = np.asarray(features, dtype=np.float32)
    labels = np.asarray(labels, dtype=np.int32)
    normal_center = np.asarray(normal_center, dtype=np.float32)
    running_sigma = np.float32(np.asarray(running_sigma))
    Bq = features.shape[0]

    stats, order, n0, _res = run_device(features, labels)
    if float((np.asarray(normal_center, np.float64) ** 2).sum()) != 0.0:
        return _finalize_general_center(stats, order, n0, labels,
                                        normal_center, running_sigma, Bq,
                                        features)
    return finalize(stats, order, n0, labels, normal_center, running_sigma, Bq)
